# revision 8
# baseline (speedup 1.0000x reference)
"""Trainium2 Bass kernel for BinarizeLinear: y = x @ sign(W).T + bias.

Full-input contract: kernel(x=[65536,1024]f32, weight=[1024,1024]f32,
bias=[1024]f32) -> y=[65536,1024]f32.

Strategy (data-parallel, 8 NeuronCores):
  - Shard the batch dim of x 8 ways (8192 rows/core); replicate the
    binarized weight and bias (per the sharding hint).
  - Host precomputes sT = sign(W).T as bf16 (+-1 exact) and pre-blocks x
    into the exact transposed SBUF tile layout, bf16 (quantization ~1e-3
    norm-relative; PSUM accumulation stays fp32). The PE then does ONLY
    matmuls: no on-chip transposes at all.
  - Per chunk of 1024 batch rows: ONE x DMA (128 descriptors x 16KB
    contiguous), 128 bf16 matmuls (K=128, N=512, 1 cycle/row) accumulated
    in fp32 PSUM, DVE bias-add eviction to bf16, ONE y DMA (interleaved
    batch order makes per-partition runs 16KB contiguous).
  - PE matmul roofline: 64 batch tiles x 16 matmuls x 512 rows
    ~= 187 us/pass; x loads (~47 us) and y stores (~47 us) overlap under.

Host x blocking: x_blk[p, c*8192 + ki*1024 + n*128 + j] =
  x_bf16[batch = c*1024 + j*8 + n, feature = ki*128 + p]
so SBUF partition p of chunk c holds 16KB contiguous; matmul lhsT slice
(ki, n) has column j <-> batch row j*8+n, making PSUM partition j hold
batch rows j*8..j*8+7 across the chunk's n-subtiles -> coalesced y DMA.
"""

from contextlib import ExitStack

import numpy as np

N_CORES = 8
B = 65536
IN_F = 1024
OUT_F = 1024
P = 128
B_SHARD = B // N_CORES  # 8192

CHUNK = 8  # batch tiles (128 rows) per chunk
KT = IN_F // P  # 8 k-tiles (contraction)
NH = OUT_F // 512  # 2 psum halves
CR = CHUNK * P  # 1024 rows per chunk
NCH = B_SHARD // CR  # 8 chunks per core

_NC_CACHE = {}


def build_nc(
    b_shard=B_SHARD,
    repeat=1,
    hw_loop=0,
    xt_bufs=3,
    y_bufs=2,
    skip_mm=False,
    skip_x=False,
):
    """Build the per-core Bass module (SPMD: same program on all cores).

    hw_loop>0 wraps the main loop in a tc.For_i hardware loop running
    hw_loop times (same I/O each iteration); repeat>1 unrolls it;
    skip_mm/skip_x drop pipeline stages — benchmarking only.
    """
    import concourse.mybir as mybir
    import concourse.tile as tile
    from concourse import bacc

    f32 = mybir.dt.float32
    bf16 = mybir.dt.bfloat16
    nch = b_shard // CR

    nc = bacc.Bacc("TRN2", target_bir_lowering=False, debug=False)
    # pre-blocked x: [128, nch * KT * CR] bf16 (see module docstring)
    x_d = nc.dram_tensor("x", [P, nch * KT * CR], bf16, kind="ExternalInput")
    # pre-blocked sign(W).T: [128, KT * OUT_F] bf16, wt[p, ki*OUT_F + o]
    # = sign(W)[o, ki*128 + p]
    wt_d = nc.dram_tensor("wt", [P, KT * OUT_F], bf16, kind="ExternalInput")
    b_d = nc.dram_tensor("bias", [1, OUT_F], f32, kind="ExternalInput")
    y_d = nc.dram_tensor("y", [b_shard, OUT_F], bf16, kind="ExternalOutput")

    with tile.TileContext(nc) as tc, ExitStack() as ctx:
        const = ctx.enter_context(tc.tile_pool(name="const", bufs=1))
        sT_pool = ctx.enter_context(tc.tile_pool(name="sT", bufs=1))
        xT_pool = ctx.enter_context(tc.tile_pool(name="xT", bufs=xt_bufs))
        y_pool = ctx.enter_context(tc.tile_pool(name="yout", bufs=y_bufs))
        mm_psum = ctx.enter_context(tc.tile_pool(name="mmp", bufs=8, space="PSUM"))

        # ---- weights: one 2MB DMA, 16KB/partition contiguous ----
        sT = sT_pool.tile([P, KT * OUT_F], bf16, name="sT")
        nc.sync.dma_start(sT[:, :], wt_d.ap()[:, :])

        # ---- bias: broadcast [1, OUT_F] -> [P, OUT_F] via a K=1 matmul ----
        bias_sb = const.tile([1, OUT_F], f32)
        nc.sync.dma_start(bias_sb[:, :], b_d.ap()[:, :])
        ones1 = const.tile([1, P], f32)
        nc.vector.memset(ones1[:, :], 1.0)
        bias_rep = const.tile([P, OUT_F], f32)
        for h in range(NH):
            bps = mm_psum.tile([P, 512], f32, tag="mm")
            nc.tensor.matmul(
                bps[:, :],
                ones1[:, :],
                bias_sb[:, h * 512 : (h + 1) * 512],
                start=True,
                stop=True,
            )
            nc.scalar.copy(bias_rep[:, h * 512 : (h + 1) * 512], bps[:, :])

        # ---- main loop over chunks of CR batch rows ----
        loop_ctx = tc.For_i(0, hw_loop, 1) if hw_loop else None
        if loop_ctx is not None:
            loop_ctx.__enter__()
        for c in [t for _ in range(repeat) for t in range(nch)]:
            xT = xT_pool.tile([P, KT * CR], bf16, tag="xT", name="xT")
            if not skip_x:
                nc.sync.dma_start(
                    xT[:, :], x_d.ap()[:, c * KT * CR : (c + 1) * KT * CR]
                )
            y_sb = y_pool.tile([P, CHUNK * OUT_F], bf16, tag="y", name="y_sb")
            for n in range(CHUNK) if not skip_mm else []:
                for h in range(NH):
                    mm = mm_psum.tile([P, 512], f32, tag="mm")
                    for ki in range(KT):
                        nc.tensor.matmul(
                            mm[:, :],
                            xT[:, ki * CR + n * P : ki * CR + (n + 1) * P],
                            sT[:, ki * OUT_F + h * 512 : ki * OUT_F + (h + 1) * 512],
                            start=(ki == 0),
                            stop=(ki == KT - 1),
                        )
                    nc.vector.tensor_add(
                        y_sb[:, n * OUT_F + h * 512 : n * OUT_F + (h + 1) * 512],
                        mm[:, :],
                        bias_rep[:, h * 512 : (h + 1) * 512],
                    )
            if skip_mm:
                nc.vector.tensor_copy(y_sb[:, :], xT[:, :])
            # partition j holds chunk rows j*CHUNK..j*CHUNK+CHUNK-1 -> per-
            # partition 16KB contiguous DRAM runs
            nc.sync.dma_start(
                y_d.ap()[c * CR : (c + 1) * CR, :].rearrange(
                    "(j n) m -> j n m", n=CHUNK
                ),
                y_sb[:, :].rearrange("p (n m) -> p n m", n=CHUNK),
            )
        if loop_ctx is not None:
            loop_ctx.__exit__(None, None, None)

    nc.compile()
    return nc


def _get_nc(b_shard=B_SHARD):
    if b_shard not in _NC_CACHE:
        _NC_CACHE[b_shard] = build_nc(b_shard)
    return _NC_CACHE[b_shard]


def _block_x(xb):
    """[shard, IN_F] bf16 -> [128, nch*KT*CR] per the module docstring."""
    shard = xb.shape[0]
    nch = shard // CR
    # batch = c*CR + j*CHUNK + n ; feature = ki*P + p
    v = xb.reshape(nch, P, CHUNK, KT, P)  # [c, j, n, ki, p]
    v = v.transpose(4, 0, 3, 2, 1)  # [p, c, ki, n, j]
    return np.ascontiguousarray(v.reshape(P, nch * KT * CR))


def make_in_maps(x, weight, bias):
    import ml_dtypes

    bf16 = ml_dtypes.bfloat16
    x = np.asarray(x, dtype=np.float32)
    weight = np.asarray(weight, dtype=np.float32)
    # sign in f32 (exact {-1,0,+1}); block: wt[p, ki*OUT_F + o] =
    # sign(W)[o, ki*128 + p] = sign(W).T[ki*128+p, o]
    st = np.sign(weight).T.astype(bf16)  # [in, out]
    wt = np.ascontiguousarray(
        st.reshape(KT, P, OUT_F).transpose(1, 0, 2).reshape(P, KT * OUT_F)
    )
    bias = np.ascontiguousarray(np.asarray(bias, dtype=np.float32)).reshape(1, OUT_F)
    shard = x.shape[0] // N_CORES
    xb = x.astype(bf16)
    return [
        {
            "x": _block_x(xb[c * shard : (c + 1) * shard]),
            "wt": wt,
            "bias": bias,
        }
        for c in range(N_CORES)
    ], shard


def run(x, weight, bias, trace=False, **kwargs):
    """Run on 8 cores; returns (y_full_f32, BassKernelResults)."""
    from concourse.bass_utils import run_bass_kernel_spmd

    in_maps, shard = make_in_maps(x, weight, bias)
    nc = _get_nc(shard)
    res = run_bass_kernel_spmd(
        nc, in_maps, core_ids=list(range(N_CORES)), trace=trace, **kwargs
    )
    y = np.concatenate(
        [np.asarray(res.results[c]["y"], dtype=np.float32) for c in range(N_CORES)],
        axis=0,
    )
    return y, res


def kernel(x, weight, bias):
    y, _ = run(x, weight, bias)
    return np.asarray(y, dtype=np.float32)


# revision 24
# speedup vs baseline: 1.0314x; 1.0314x over previous
"""Trainium2 Bass kernel for BinarizeLinear: y = x @ sign(W).T + bias.

Full-input contract: kernel(x=[65536,1024]f32, weight=[1024,1024]f32,
bias=[1024]f32) -> y=[65536,1024]f32.

Strategy (data-parallel, 8 NeuronCores):
  - Shard the batch dim of x 8 ways (8192 rows/core); replicate the
    binarized weight and bias (per the sharding hint).
  - Host precomputes sT = sign(W).T as bf16 (+-1 exact) and pre-blocks x
    into the exact transposed SBUF tile layout, bf16 (quantization ~1e-3
    norm-relative; PSUM accumulation stays fp32). The PE then does ONLY
    matmuls: no on-chip transposes at all.
  - Per chunk of 1024 batch rows: x loads in 4 split DMAs (16KB/partition
    contiguous total, finer-grained overlap), 128 bf16 matmuls (K=128,
    N=512) accumulated in fp32 PSUM, DVE bias-add eviction to bf16, ONE
    y DMA (interleaved batch order -> 16KB contiguous per-partition runs).
  - Measured steady state ~240-275 us/pass (device-state dependent),
    vs ~360-380 us for the previous f32r + PE-transpose pipeline in the
    same measurement windows; pure-matmul floor measured ~235 us.

Host x blocking: x_blk[p, c*8192 + ki*1024 + n*128 + j] =
  x_bf16[batch = c*1024 + j*8 + n, feature = ki*128 + p]
so SBUF partition p of chunk c holds 16KB contiguous; matmul lhsT slice
(ki, n) has column j <-> batch row j*8+n, making PSUM partition j hold
batch rows j*8..j*8+7 across the chunk's n-subtiles -> coalesced y DMA.
"""

from contextlib import ExitStack

import numpy as np

N_CORES = 8
B = 65536
IN_F = 1024
OUT_F = 1024
P = 128
B_SHARD = B // N_CORES  # 8192

CHUNK = 8  # batch tiles (128 rows) per chunk
KT = IN_F // P  # 8 k-tiles (contraction)
NH = OUT_F // 512  # 2 psum halves
CR = CHUNK * P  # 1024 rows per chunk
NCH = B_SHARD // CR  # 8 chunks per core

# matmul operand dtype: "f32r" (tf32, x ships as raw f32 bits),
# "fp16", or "bf16" (2-byte dtypes, x quantized on host)
MM_DT = "bf16"

_NC_CACHE = {}


def _np_op_dt(mm_dt):
    import ml_dtypes

    return {
        "f32r": np.float32,
        "fp16": np.float16,
        "bf16": ml_dtypes.bfloat16,
    }[mm_dt]


def build_nc(
    b_shard=B_SHARD,
    repeat=1,
    hw_loop=0,
    xt_bufs=3,
    y_bufs=2,
    skip_mm=False,
    skip_x=False,
    skip_evict=False,
    evict="dve",
    mm_dt=None,
    x_split=4,
):
    """Build the per-core Bass module (SPMD: same program on all cores).

    hw_loop>0 wraps the main loop in a tc.For_i hardware loop running
    hw_loop times (same I/O each iteration); repeat>1 unrolls it;
    skip_mm/skip_x drop pipeline stages — benchmarking only.
    """
    import concourse.mybir as mybir
    import concourse.tile as tile
    from concourse import bacc

    if mm_dt is None:
        mm_dt = MM_DT
    f32 = mybir.dt.float32
    bf16 = mybir.dt.bfloat16
    op = {
        "f32r": mybir.dt.float32r,
        "fp16": mybir.dt.float16,
        "bf16": bf16,
    }[mm_dt]
    nch = b_shard // CR

    nc = bacc.Bacc("TRN2", target_bir_lowering=False, debug=False)
    # pre-blocked x: [128, nch * KT * CR] (see module docstring)
    x_d = nc.dram_tensor("x", [P, nch * KT * CR], op, kind="ExternalInput")
    # pre-blocked sign(W).T: [128, KT * OUT_F], wt[p, ki*OUT_F + o]
    # = sign(W)[o, ki*128 + p]
    wt_d = nc.dram_tensor("wt", [P, KT * OUT_F], op, kind="ExternalInput")
    b_d = nc.dram_tensor("bias", [1, OUT_F], f32, kind="ExternalInput")
    y_d = nc.dram_tensor("y", [b_shard, OUT_F], bf16, kind="ExternalOutput")

    with tile.TileContext(nc) as tc, ExitStack() as ctx:
        const = ctx.enter_context(tc.tile_pool(name="const", bufs=1))
        sT_pool = ctx.enter_context(tc.tile_pool(name="sT", bufs=1))
        xT_pool = ctx.enter_context(tc.tile_pool(name="xT", bufs=xt_bufs))
        y_pool = ctx.enter_context(tc.tile_pool(name="yout", bufs=y_bufs))
        mm_psum = ctx.enter_context(tc.tile_pool(name="mmp", bufs=8, space="PSUM"))

        # ---- weights: one 2MB DMA, 16KB/partition contiguous ----
        sT = sT_pool.tile([P, KT * OUT_F], op, name="sT")
        nc.sync.dma_start(sT[:, :], wt_d.ap()[:, :])

        # ---- bias: broadcast [1, OUT_F] -> [P, OUT_F] via a K=1 matmul ----
        bias_sb = const.tile([1, OUT_F], f32)
        nc.sync.dma_start(bias_sb[:, :], b_d.ap()[:, :])
        ones1 = const.tile([1, P], f32)
        nc.vector.memset(ones1[:, :], 1.0)
        bias_rep = const.tile([P, OUT_F], f32)
        for h in range(NH):
            bps = mm_psum.tile([P, 512], f32, tag="mm")
            nc.tensor.matmul(
                bps[:, :],
                ones1[:, :],
                bias_sb[:, h * 512 : (h + 1) * 512],
                start=True,
                stop=True,
            )
            nc.scalar.copy(bias_rep[:, h * 512 : (h + 1) * 512], bps[:, :])

        # ---- main loop over chunks of CR batch rows ----
        xT_static = None
        if skip_x:
            # DMA chunk 0 once (DMA is a sanctioned producer for any dtype)
            xT_static = const.tile([P, KT * CR], op, name="xTs")
            nc.sync.dma_start(xT_static[:, :], x_d.ap()[:, : KT * CR])
        y_static = None
        if skip_evict:
            y_static = const.tile([P, CHUNK * OUT_F], bf16, name="ys")
            nc.vector.memset(y_static[:, :], 0.0)
        loop_ctx = tc.For_i(0, hw_loop, 1) if hw_loop else None
        if loop_ctx is not None:
            loop_ctx.__enter__()
        for c in [t for _ in range(repeat) for t in range(nch)]:
            if skip_x:
                xT = xT_static
            else:
                xT = xT_pool.tile([P, KT * CR], op, tag="xT", name="xT")
                # split the chunk load into x_split DMAs (finer-grained
                # overlap: k-chains can start when their slice lands)
                sw = KT * CR // x_split
                for s in range(x_split):
                    nc.sync.dma_start(
                        xT[:, s * sw : (s + 1) * sw],
                        x_d.ap()[
                            :, c * KT * CR + s * sw : c * KT * CR + (s + 1) * sw
                        ],
                    )
            y_sb = (
                y_static
                if skip_evict
                else y_pool.tile([P, CHUNK * OUT_F], bf16, tag="y", name="y_sb")
            )
            for n in range(CHUNK) if not skip_mm else []:
                for h in range(NH):
                    mm = mm_psum.tile([P, 512], f32, tag="mm")
                    for ki in range(KT):
                        nc.tensor.matmul(
                            mm[:, :],
                            xT[:, ki * CR + n * P : ki * CR + (n + 1) * P],
                            sT[:, ki * OUT_F + h * 512 : ki * OUT_F + (h + 1) * 512],
                            start=(ki == 0),
                            stop=(ki == KT - 1),
                        )
                    if not skip_evict:
                        ev = nc.vector if evict == "dve" or (
                            evict == "split" and (n * NH + h) % 2 == 0
                        ) else nc.gpsimd
                        ev.tensor_add(
                            y_sb[:, n * OUT_F + h * 512 : n * OUT_F + (h + 1) * 512],
                            mm[:, :],
                            bias_rep[:, h * 512 : (h + 1) * 512],
                        )
            if skip_mm and not skip_evict:
                nc.vector.tensor_copy(y_sb[:, :], xT[:, :])
            # partition j holds chunk rows j*CHUNK..j*CHUNK+CHUNK-1 -> per-
            # partition 16KB contiguous DRAM runs
            nc.sync.dma_start(
                y_d.ap()[c * CR : (c + 1) * CR, :].rearrange(
                    "(j n) m -> j n m", n=CHUNK
                ),
                y_sb[:, :].rearrange("p (n m) -> p n m", n=CHUNK),
            )
        if loop_ctx is not None:
            loop_ctx.__exit__(None, None, None)

    nc.compile()
    return nc


def _get_nc(b_shard=B_SHARD):
    if b_shard not in _NC_CACHE:
        _NC_CACHE[b_shard] = build_nc(b_shard)
    return _NC_CACHE[b_shard]


def _block_x(xb):
    """[shard, IN_F] bf16 -> [128, nch*KT*CR] per the module docstring."""
    shard = xb.shape[0]
    nch = shard // CR
    # batch = c*CR + j*CHUNK + n ; feature = ki*P + p
    v = xb.reshape(nch, P, CHUNK, KT, P)  # [c, j, n, ki, p]
    v = v.transpose(4, 0, 3, 2, 1)  # [p, c, ki, n, j]
    return np.ascontiguousarray(v.reshape(P, nch * KT * CR))


def make_in_maps(x, weight, bias, mm_dt=None):
    if mm_dt is None:
        mm_dt = MM_DT
    np_dt = _np_op_dt(mm_dt)
    x = np.asarray(x, dtype=np.float32)
    weight = np.asarray(weight, dtype=np.float32)
    # sign in f32 (exact {-1,0,+1}); block: wt[p, ki*OUT_F + o] =
    # sign(W)[o, ki*128 + p] = sign(W).T[ki*128+p, o]
    st = np.sign(weight).T.astype(np_dt)  # [in, out]
    wt = np.ascontiguousarray(
        st.reshape(KT, P, OUT_F).transpose(1, 0, 2).reshape(P, KT * OUT_F)
    )
    bias = np.ascontiguousarray(np.asarray(bias, dtype=np.float32)).reshape(1, OUT_F)
    shard = x.shape[0] // N_CORES
    xb = x.astype(np_dt)
    return [
        {
            "x": _block_x(xb[c * shard : (c + 1) * shard]),
            "wt": wt,
            "bias": bias,
        }
        for c in range(N_CORES)
    ], shard


def run(x, weight, bias, trace=False, **kwargs):
    """Run on 8 cores; returns (y_full_f32, BassKernelResults)."""
    from concourse.bass_utils import run_bass_kernel_spmd

    in_maps, shard = make_in_maps(x, weight, bias)
    nc = _get_nc(shard)
    res = run_bass_kernel_spmd(
        nc, in_maps, core_ids=list(range(N_CORES)), trace=trace, **kwargs
    )
    y = np.concatenate(
        [np.asarray(res.results[c]["y"], dtype=np.float32) for c in range(N_CORES)],
        axis=0,
    )
    return y, res


def kernel(x, weight, bias):
    y, _ = run(x, weight, bias)
    return np.asarray(y, dtype=np.float32)
